# revision 23
# baseline (speedup 1.0000x reference)
"""Trainium2 Bass kernel for DecoderRNN (embed -> LSTM -> vocab FC).

Strategy (8 NeuronCores, SPMD):
  - Embedding gather, x_proj precompute and the LSTM recurrence are
    replicated on every core (per-step cross-core collectives are
    latency-bound and slower than replicating ~7us/step of matmul).
  - The dominant FC (hs @ fc_W.T, 134 of 160 GFLOP) is sharded along the
    vocab dim: each core gets 4000 rows of fc_W (zero-padded to 4096) and
    produces logits[:, shard]. Host concatenates.
  - All weight matrices are pre-transposed/cast to bf16 on the host into
    the [K-tile, 128, N] layouts the PE consumes; only the data-dependent
    embedding gather is transposed on device (via bf16 DMA-transpose).
  - Token order is t-major everywhere (bt = t*64 + b).
  - The recurrence matmul is column-tiled: batch (M=64) fills PE columns
    0:63 for gate cols [i|f] and columns 64:127 for [g|o] concurrently,
    one single-bank PSUM tile per 512-slice; gates are slice-pipelined
    under the matmul stream (ACT moves the g|o half PSUM->SBUF for free).
  - x_proj m-tiles and FC pieces are woven between LSTM steps to keep the
    PE busy through the recurrence's gate latency.
  - b_ih+b_hh folded into the encoder projection via K-augmentation.
"""

import os
import sys

import numpy as np

for _p in ("/opt/trn_rl_repo", "/root/.axon_site/_ro/trn_rl_repo"):
    if os.path.isdir(_p) and _p not in sys.path:
        sys.path.append(_p)

import ml_dtypes
import concourse.bass as bass
import concourse.mybir as mybir
from concourse import bacc
import concourse.tile as tile
from concourse.bass_utils import run_bass_kernel_spmd

F32 = mybir.dt.float32
BF16 = mybir.dt.bfloat16
I32 = mybir.dt.int32
BF16NP = ml_dtypes.bfloat16

B, T = 64, 32
E, H, V = 512, 1024, 32000
G4 = 4 * H          # 4096
BT = B * T          # 2048
EA = E + 128        # augmented enc K dim (ones col + zero pad)
NCORES = 8
VL = V // NCORES    # 4000 real vocab rows per core
VLP = 4096          # padded vocab rows per core

Sig = mybir.ActivationFunctionType.Sigmoid
Tanh = mybir.ActivationFunctionType.Tanh

_nc_cache = None


def build_nc() -> bass.Bass:
    nc = bacc.Bacc()

    idx = nc.declare_dram_parameter("idx", [16, 128, 1], I32, isOutput=False)
    emb = nc.declare_dram_parameter("emb", [V, E], F32, isOutput=False)
    c0 = nc.declare_dram_parameter("c0", [B, H], F32, isOutput=False)
    h0T = nc.declare_dram_parameter("h0T", [8, 128, B], BF16, isOutput=False)
    encT = nc.declare_dram_parameter("encT", [5, 128, B], BF16, isOutput=False)
    wihT = nc.declare_dram_parameter("wihT", [4, 128, G4], BF16, isOutput=False)
    wencT = nc.declare_dram_parameter("wencT", [5, 128, G4], BF16, isOutput=False)
    whhT = nc.declare_dram_parameter("whhT", [8, 128, G4], BF16, isOutput=False)
    fcwT = nc.declare_dram_parameter("fcwT", [8, 128, VLP], BF16, isOutput=False)
    fcb = nc.declare_dram_parameter("fcb", [128, VLP], BF16, isOutput=False)

    logits = nc.declare_dram_parameter("logits", [BT, VLP], F32, isOutput=True)
    hn = nc.declare_dram_parameter("hn", [B, H], F32, isOutput=True)
    cn = nc.declare_dram_parameter("cn", [B, H], F32, isOutput=True)

    with tile.TileContext(nc) as tc:
        with (
            tc.tile_pool(name="dram", bufs=1, space="DRAM") as dr,
            tc.tile_pool(name="constp", bufs=1) as constp,
            tc.tile_pool(name="wres", bufs=1) as wres,
            tc.tile_pool(name="st1", bufs=1) as st1,
            tc.tile_pool(name="st2", bufs=1) as st2,
            tc.tile_pool(name="statep", bufs=1) as statep,
            tc.tile_pool(name="pz", bufs=1, space="PSUM") as pz,
            tc.tile_pool(name="pfc", bufs=4, space="PSUM") as pfc,
        ):
            # ---- DRAM scratch
            emb_bf = [
                dr.tile([128, E], BF16, name=f"emb_bf{m}") for m in range(16)
            ]
            xpe = [dr.tile([128, 2048], BF16, name=f"xpe{t}") for t in range(T)]
            hsT_d = [
                dr.tile([128, 8, 128], BF16, name=f"hsT_d{m}") for m in range(16)
            ]

            # ---- embedding gather (t-major), cast bf16, transpose
            for m in range(16):
                idx_sb = st2.tile([128, 1], I32, tag="idx", bufs=2)
                nc.sync.dma_start(idx_sb[:], idx[m])
                g_f = st2.tile([128, E], F32, tag="gf", bufs=1)
                nc.gpsimd.indirect_dma_start(
                    out=g_f[:],
                    out_offset=None,
                    in_=emb[:],
                    in_offset=bass.IndirectOffsetOnAxis(ap=idx_sb[:, :1], axis=0),
                )
                g_b = st2.tile([128, E], BF16, tag="gb", bufs=2)
                nc.vector.tensor_copy(g_b[:], g_f[:])
                nc.sync.dma_start(emb_bf[m][:], g_b[:])
            embT = []
            for k in range(4):
                tT = st1.tile([128, BT], BF16, tag=f"embT{k}", name=f"embT{k}")
                embT.append(tT)
            for m in range(16):
                for k in range(4):
                    nc.sync.dma_start_transpose(
                        embT[k][:, m * 128 : (m + 1) * 128],
                        emb_bf[m][:, k * 128 : (k + 1) * 128],
                    )

            # ---- resident weights (direct loads of host-prepped layouts)
            Ww = []
            for k in range(4):
                tT = st1.tile([128, G4], BF16, tag=f"Ww{k}", name=f"Ww{k}")
                nc.scalar.dma_start(tT[:], wihT[k])
                Ww.append(tT)
            WhT = []
            for k in range(8):
                tT = wres.tile([128, G4], BF16, tag=f"WhT{k}", name=f"WhT{k}")
                nc.scalar.dma_start(tT[:], whhT[k])
                WhT.append(tT)
            fwA = []
            for k in range(8):
                fwk = st1.tile([128, 512], BF16, tag=f"fwA{k}", name=f"fwA{k}")
                nc.gpsimd.dma_start(fwk[:], fcwT[k][:, 0:512])
                fwA.append(fwk)
            fcb_sb = constp.tile([128, VLP], BF16)
            nc.scalar.dma_start(fcb_sb[:], fcb[:])

            # ---- enc_proj (+bias via augmentation) -> enc_exp2 bf16
            encT_sb = []
            for k in range(5):
                tT = st2.tile([128, B], BF16, tag=f"encT{k}", name=f"encT{k}")
                nc.scalar.dma_start(tT[:], encT[k])
                encT_sb.append(tT)
            enc_exp2 = wres.tile([128, G4], BF16)
            for half in range(2):
                hoff = half * 2048
                ps_es = [
                    pfc.tile([64, 512], F32, tag="fc", name=f"ps_e{n}")
                    for n in range(4)
                ]
                for k in range(5):
                    weT = st2.tile([128, 2048], BF16, tag="WeT", bufs=2)
                    nc.scalar.dma_start(weT[:], wencT[k][:, hoff : hoff + 2048])
                    for n in range(4):
                        nc.tensor.matmul(
                            ps_es[n][:],
                            lhsT=encT_sb[k][:],
                            rhs=weT[:, n * 512 : (n + 1) * 512],
                            start=(k == 0),
                            stop=(k == 4),
                        )
                for n in range(4):
                    nc.vector.tensor_copy(
                        enc_exp2[0:64, hoff + n * 512 : hoff + (n + 1) * 512],
                        ps_es[n][:],
                    )
            nc.sync.dma_start(enc_exp2[64:128], enc_exp2[0:64])

            # ---- program pieces
            def xproj_m(m):
                """x_proj for tokens t0=2m, t1=2m+1 -> xpe[t0], xpe[t1]."""
                t0, t1 = 2 * m, 2 * m + 1
                for half in range(2):
                    xrow = slice(0, 64) if half == 0 else slice(64, 128)
                    for n in range(4):
                        co = half * 2048 + n * 512
                        ps_x = pfc.tile([128, 512], F32, tag="fc", name="ps_x")
                        for k in range(4):
                            nc.tensor.matmul(
                                ps_x[:],
                                lhsT=embT[k][:, m * 128 : (m + 1) * 128],
                                rhs=Ww[k][:, co : co + 512],
                                start=(k == 0),
                                stop=(k == 3),
                            )
                        x_sb = st2.tile([128, 512], BF16, tag="xsb", bufs=4)
                        nc.vector.tensor_add(
                            x_sb[:], ps_x[:], enc_exp2[:, co : co + 512]
                        )
                        ns = slice(n * 512, (n + 1) * 512)
                        nc.scalar.dma_start(xpe[t0][xrow, ns], x_sb[0:64])
                        nc.scalar.dma_start(xpe[t1][xrow, ns], x_sb[64:128])

            def fc_piece(m, vc, fw_row):
                v0 = vc * 512
                hsL = st2.tile([128, 1024], BF16, tag="hsL", bufs=4)
                nc.gpsimd.dma_start(
                    hsL[:], hsT_d[m][:].rearrange("p k x -> p (k x)")
                )
                ps_f = pfc.tile([128, 512], F32, tag="fc", name="ps_f")
                for k in range(8):
                    nc.tensor.matmul(
                        ps_f[:],
                        lhsT=hsL[:, k * 128 : (k + 1) * 128],
                        rhs=fw_row[k][:],
                        start=(k == 0),
                        stop=(k == 7),
                    )
                o_sb = st2.tile([128, 512], F32, tag="fco", bufs=2)
                nc.vector.tensor_add(o_sb[:], ps_f[:], fcb_sb[:, v0 : v0 + 512])
                nc.scalar.dma_start(
                    logits[m * 128 : (m + 1) * 128, v0 : v0 + 512], o_sb[:]
                )

            # ---- LSTM with woven fillers
            h_sb = statep.tile([B, H], F32, tag="h")
            c_sb = statep.tile([B, H], F32, tag="c")
            nc.sync.dma_start(c_sb[:], c0[:])
            hT_prev = st2.tile([128, 512], BF16, tag="hT", bufs=2)
            for k in range(8):
                nc.scalar.dma_start(hT_prev[:, k * 64 : (k + 1) * 64], h0T[k])

            ident64 = constp.tile([64, 64], F32)
            from concourse.masks import make_identity

            make_identity(nc, ident64[:])

            xproj_m(0)
            xproj_m(1)
            xproj_m(2)
            next_fc_m = 0

            for t in range(T):
                xp = st2.tile([128, 2048], BF16, tag="xp", bufs=2)
                nc.sync.dma_start(xp[:], xpe[t][:])

                zt = []
                for n in range(4):
                    ps_zn = pz.tile([128, 512], F32, tag=f"z{n}", name=f"ps_zn{n}")
                    for k in range(8):
                        lhsT = hT_prev[:, k * 64 : (k + 1) * 64]
                        nc.tensor.matmul(
                            ps_zn[0:64, :],
                            lhsT=lhsT,
                            rhs=WhT[k][:, n * 512 : (n + 1) * 512],
                            start=(k == 0),
                            stop=(k == 7),
                            tile_position=(0, 0),
                            skip_group_check=True,
                        )
                        nc.tensor.matmul(
                            ps_zn[64:128, :],
                            lhsT=lhsT,
                            rhs=WhT[k][:, 2048 + n * 512 : 2048 + (n + 1) * 512],
                            start=(k == 0),
                            stop=(k == 7),
                            tile_position=(0, 64),
                            skip_group_check=True,
                        )
                    ns = slice(n * 512, (n + 1) * 512)
                    nc.vector.tensor_add(ps_zn[:], ps_zn[:], xp[:, ns])
                    nc.scalar.activation(ps_zn[0:64, :], ps_zn[0:64, :], Sig)
                    if n == 0:
                        go_sb = statep.tile([B, 2048], F32, tag="go", name="go_sb")
                    nc.scalar.activation(
                        go_sb[:, ns], ps_zn[64:128, :], Tanh if n < 2 else Sig
                    )
                    zt.append(ps_zn)

                # PE fillers while the c/h tail runs
                filled = False
                if t % 2 == 1:
                    mx = 3 + (t - 1) // 2
                    if mx <= 15:
                        xproj_m(mx)
                        filled = True
                if not filled and t >= 4 and next_fc_m <= min(15, (t - 2) // 2):
                    fc_piece(next_fc_m, 0, fwA)
                    next_fc_m += 1

                ps_tr = pfc.tile([128, 512], F32, tag="fc", name="ps_tr")
                for j in range(2):
                    js = slice(j * 512, (j + 1) * 512)
                    nc.vector.tensor_mul(c_sb[:, js], zt[2 + j][0:64, :], c_sb[:, js])
                    nc.vector.tensor_mul(go_sb[:, js], zt[j][0:64, :], go_sb[:, js])
                    nc.vector.tensor_add(c_sb[:, js], c_sb[:, js], go_sb[:, js])
                    nc.scalar.activation(go_sb[:, js], c_sb[:, js], Tanh)
                    nc.vector.tensor_mul(
                        h_sb[:, js],
                        go_sb[:, (2 + j) * 512 : (3 + j) * 512],
                        go_sb[:, js],
                    )
                    for k in range(4 * j, 4 * j + 4):
                        nc.tensor.transpose(
                            out=ps_tr[:, k * 64 : (k + 1) * 64],
                            in_=h_sb[:, k * 128 : (k + 1) * 128],
                            identity=ident64[:],
                        )
                hT_cur = st2.tile([128, 512], BF16, tag="hT", bufs=2)
                nc.vector.tensor_copy(hT_cur[:], ps_tr[:])
                m, hlf = t // 2, t % 2
                for k in range(8):
                    nc.sync.dma_start(
                        hsT_d[m][:, k, hlf * 64 : (hlf + 1) * 64],
                        hT_cur[:, k * 64 : (k + 1) * 64],
                    )
                hT_prev = hT_cur

            nc.sync.dma_start(hn[:], h_sb[:])
            nc.sync.dma_start(cn[:], c_sb[:])

            # ---- remaining FC: finish vc0, then chunks 1..7
            while next_fc_m <= 15:
                fc_piece(next_fc_m, 0, fwA)
                next_fc_m += 1
            for vc in range(1, 8):
                v0 = vc * 512
                fw = []
                for k in range(8):
                    fwk = st1.tile(
                        [128, 512], BF16, tag=f"fw{k}", name=f"fw{vc}_{k}", bufs=2
                    )
                    nc.gpsimd.dma_start(fwk[:], fcwT[k][:, v0 : v0 + 512])
                    fw.append(fwk)
                for m in range(16):
                    fc_piece(m, vc, fw)

    nc.compile()
    return nc


def get_nc() -> bass.Bass:
    global _nc_cache
    if _nc_cache is None:
        _nc_cache = build_nc()
    return _nc_cache


def _kt(a, k):
    """[K, N] -> [K/128, 128, N] bf16 contiguous."""
    K = a.shape[0]
    return np.ascontiguousarray(
        a.reshape(K // 128, 128, a.shape[1]).astype(BF16NP)
    )


def make_in_maps(inputs_np: dict) -> list:
    inp = {k: np.asarray(v) for k, v in inputs_np.items()}
    idx_tm = (
        np.ascontiguousarray(inp["inputs"].astype(np.int32).T)
        .reshape(16, 128, 1)
        .copy()
    )
    W_ih = np.asarray(inp["W_ih"], dtype=np.float32)
    bias = (
        np.asarray(inp["b_ih"], dtype=np.float32)
        + np.asarray(inp["b_hh"], dtype=np.float32)
    )
    enc_aug = np.zeros((EA, B), dtype=np.float32)
    enc_aug[:E] = np.asarray(inp["encoder_features"], dtype=np.float32).T
    enc_aug[E] = 1.0
    wenc_augT = np.zeros((EA, G4), dtype=np.float32)
    wenc_augT[:E] = W_ih[:, E:].T
    wenc_augT[E] = bias
    h0T = np.ascontiguousarray(np.asarray(inp["h0"], dtype=np.float32).T)
    common = dict(
        idx=idx_tm,
        emb=np.ascontiguousarray(inp["embed_table"], dtype=np.float32),
        c0=np.ascontiguousarray(inp["c0"], dtype=np.float32),
        h0T=_kt(h0T, 8),
        encT=_kt(enc_aug, 5),
        wihT=_kt(np.ascontiguousarray(W_ih[:, :E].T), 4),
        wencT=_kt(wenc_augT, 5),
        whhT=_kt(np.ascontiguousarray(np.asarray(inp["W_hh"], np.float32).T), 8),
    )
    fc_W = np.asarray(inp["fc_W"], dtype=np.float32)
    fc_b = np.asarray(inp["fc_b"], dtype=np.float32)
    in_maps = []
    for i in range(NCORES):
        fcw_pad = np.zeros((H, VLP), dtype=np.float32)
        fcw_pad[:, :VL] = fc_W[i * VL : (i + 1) * VL].T
        fcb_pad = np.zeros((1, VLP), dtype=np.float32)
        fcb_pad[0, :VL] = fc_b[i * VL : (i + 1) * VL]
        fcb_rep = np.broadcast_to(fcb_pad, (128, VLP)).astype(BF16NP)
        in_maps.append(
            dict(common, fcwT=_kt(fcw_pad, 8), fcb=np.ascontiguousarray(fcb_rep))
        )
    return in_maps


def assemble(results: list):
    logits_tm = np.concatenate(
        [results[i]["logits"][:, :VL] for i in range(NCORES)], axis=1
    )  # [BT(t-major), V]
    logits = np.ascontiguousarray(
        logits_tm.reshape(T, B, V).transpose(1, 0, 2)
    )  # [B, T, V]
    hn = results[0]["hn"]
    cn = results[0]["cn"]
    return logits, hn, cn


def run(inputs_np: dict, trace: bool = False):
    nc = get_nc()
    in_maps = make_in_maps(inputs_np)
    res = run_bass_kernel_spmd(
        nc, in_maps, core_ids=list(range(NCORES)), trace=trace
    )
    return assemble(res.results), res


def kernel(**inputs) -> tuple:
    (logits, hn, cn), _ = run(inputs, trace=False)
    return logits, hn, cn


# revision 24
# speedup vs baseline: 1.0269x; 1.0269x over previous
"""Trainium2 Bass kernel for DecoderRNN (embed -> LSTM -> vocab FC).

Strategy (8 NeuronCores, SPMD):
  - Embedding gather, x_proj precompute and the LSTM recurrence are
    replicated on every core (per-step cross-core collectives are
    latency-bound and slower than replicating ~7us/step of matmul).
  - The dominant FC (hs @ fc_W.T, 134 of 160 GFLOP) is sharded along the
    vocab dim: each core gets 4000 rows of fc_W (zero-padded to 4096) and
    produces logits[:, shard]. Host concatenates.
  - All weight matrices are pre-transposed/cast to bf16 on the host into
    the [K-tile, 128, N] layouts the PE consumes; only the data-dependent
    embedding gather is transposed on device (via bf16 DMA-transpose).
  - Token order is t-major everywhere (bt = t*64 + b).
  - The recurrence matmul is column-tiled: batch (M=64) fills PE columns
    0:63 for gate cols [i|f] and columns 64:127 for [g|o] concurrently,
    one single-bank PSUM tile per 512-slice; gates are slice-pipelined
    under the matmul stream (ACT moves the g|o half PSUM->SBUF for free).
  - x_proj m-tiles and FC pieces are woven between LSTM steps to keep the
    PE busy through the recurrence's gate latency.
  - b_ih+b_hh folded into the encoder projection via K-augmentation.
"""

import os
import sys

import numpy as np

for _p in ("/opt/trn_rl_repo", "/root/.axon_site/_ro/trn_rl_repo"):
    if os.path.isdir(_p) and _p not in sys.path:
        sys.path.append(_p)

import ml_dtypes
import concourse.bass as bass
import concourse.mybir as mybir
from concourse import bacc
import concourse.tile as tile
from concourse.bass_utils import run_bass_kernel_spmd

F32 = mybir.dt.float32
BF16 = mybir.dt.bfloat16
I32 = mybir.dt.int32
BF16NP = ml_dtypes.bfloat16

B, T = 64, 32
E, H, V = 512, 1024, 32000
G4 = 4 * H          # 4096
BT = B * T          # 2048
EA = E + 128        # augmented enc K dim (ones col + zero pad)
NCORES = 8
VL = V // NCORES    # 4000 real vocab rows per core
VLP = 4096          # padded vocab rows per core

Sig = mybir.ActivationFunctionType.Sigmoid
Tanh = mybir.ActivationFunctionType.Tanh

_nc_cache = None


def build_nc() -> bass.Bass:
    nc = bacc.Bacc()

    idx = nc.declare_dram_parameter("idx", [16, 128, 1], I32, isOutput=False)
    emb = nc.declare_dram_parameter("emb", [V, E], F32, isOutput=False)
    c0 = nc.declare_dram_parameter("c0", [B, H], F32, isOutput=False)
    h0T = nc.declare_dram_parameter("h0T", [8, 128, B], BF16, isOutput=False)
    encT = nc.declare_dram_parameter("encT", [5, 128, B], BF16, isOutput=False)
    wihT = nc.declare_dram_parameter("wihT", [4, 128, G4], BF16, isOutput=False)
    wencT = nc.declare_dram_parameter("wencT", [5, 128, G4], BF16, isOutput=False)
    whhT = nc.declare_dram_parameter("whhT", [8, 128, G4], BF16, isOutput=False)
    fcwT = nc.declare_dram_parameter("fcwT", [8, 128, VLP], BF16, isOutput=False)
    fcb = nc.declare_dram_parameter("fcb", [128, VLP], BF16, isOutput=False)

    logits = nc.declare_dram_parameter("logits", [BT, VLP], F32, isOutput=True)
    hn = nc.declare_dram_parameter("hn", [B, H], F32, isOutput=True)
    cn = nc.declare_dram_parameter("cn", [B, H], F32, isOutput=True)

    with tile.TileContext(nc) as tc:
        with (
            tc.tile_pool(name="dram", bufs=1, space="DRAM") as dr,
            tc.tile_pool(name="constp", bufs=1) as constp,
            tc.tile_pool(name="wres", bufs=1) as wres,
            tc.tile_pool(name="st1", bufs=1) as st1,
            tc.tile_pool(name="st2", bufs=1) as st2,
            tc.tile_pool(name="statep", bufs=1) as statep,
            tc.tile_pool(name="pz", bufs=1, space="PSUM") as pz,
            tc.tile_pool(name="pfc", bufs=4, space="PSUM") as pfc,
        ):
            # ---- DRAM scratch
            emb_bf = [
                dr.tile([128, E], BF16, name=f"emb_bf{m}") for m in range(16)
            ]
            xpe = [dr.tile([128, 2048], BF16, name=f"xpe{t}") for t in range(T)]
            hsT_d = [
                dr.tile([128, 8, 128], BF16, name=f"hsT_d{m}") for m in range(16)
            ]

            # ---- embedding gather (t-major), cast bf16, transpose
            for m in range(16):
                idx_sb = st2.tile([128, 1], I32, tag="idx", bufs=2)
                nc.sync.dma_start(idx_sb[:], idx[m])
                g_f = st2.tile([128, E], F32, tag="gf", bufs=1)
                nc.gpsimd.indirect_dma_start(
                    out=g_f[:],
                    out_offset=None,
                    in_=emb[:],
                    in_offset=bass.IndirectOffsetOnAxis(ap=idx_sb[:, :1], axis=0),
                )
                g_b = st2.tile([128, E], BF16, tag="gb", bufs=2)
                nc.vector.tensor_copy(g_b[:], g_f[:])
                nc.sync.dma_start(emb_bf[m][:], g_b[:])
            embT = []
            for k in range(4):
                tT = st1.tile([128, BT], BF16, tag=f"embT{k}", name=f"embT{k}")
                embT.append(tT)
            for m in range(16):
                for k in range(4):
                    nc.sync.dma_start_transpose(
                        embT[k][:, m * 128 : (m + 1) * 128],
                        emb_bf[m][:, k * 128 : (k + 1) * 128],
                    )

            # ---- enc_proj (+bias via augmentation) -> enc_exp2 bf16
            encT_sb = []
            for k in range(5):
                tT = st2.tile([128, B], BF16, tag=f"encT{k}", name=f"encT{k}")
                nc.scalar.dma_start(tT[:], encT[k])
                encT_sb.append(tT)
            enc_exp2 = wres.tile([128, G4], BF16)
            for half in range(2):
                hoff = half * 2048
                ps_es = [
                    pfc.tile([64, 512], F32, tag="fc", name=f"ps_e{n}")
                    for n in range(4)
                ]
                for k in range(5):
                    weT = st2.tile([128, 2048], BF16, tag="WeT", bufs=2)
                    nc.scalar.dma_start(weT[:], wencT[k][:, hoff : hoff + 2048])
                    for n in range(4):
                        nc.tensor.matmul(
                            ps_es[n][:],
                            lhsT=encT_sb[k][:],
                            rhs=weT[:, n * 512 : (n + 1) * 512],
                            start=(k == 0),
                            stop=(k == 4),
                        )
                for n in range(4):
                    nc.vector.tensor_copy(
                        enc_exp2[0:64, hoff + n * 512 : hoff + (n + 1) * 512],
                        ps_es[n][:],
                    )
            nc.sync.dma_start(enc_exp2[64:128], enc_exp2[0:64])

            # ---- resident weights, spread across DMA engines
            engs = [nc.sync, nc.scalar, nc.gpsimd]
            Ww = []
            for k in range(4):
                tT = st1.tile([128, G4], BF16, tag=f"Ww{k}", name=f"Ww{k}")
                engs[k % 2].dma_start(tT[:], wihT[k])
                Ww.append(tT)
            WhT = []
            for k in range(8):
                tT = wres.tile([128, G4], BF16, tag=f"WhT{k}", name=f"WhT{k}")
                engs[k % 3].dma_start(tT[:], whhT[k])
                WhT.append(tT)
            fwA = []
            for k in range(8):
                fwk = st1.tile([128, 512], BF16, tag=f"fwA{k}", name=f"fwA{k}")
                nc.gpsimd.dma_start(fwk[:], fcwT[k][:, 0:512])
                fwA.append(fwk)
            fcb_sb = constp.tile([128, VLP], BF16)
            nc.scalar.dma_start(fcb_sb[:], fcb[:])

            # ---- program pieces
            def xproj_m(m):
                """x_proj for tokens t0=2m, t1=2m+1 -> xpe[t0], xpe[t1]."""
                t0, t1 = 2 * m, 2 * m + 1
                for half in range(2):
                    xrow = slice(0, 64) if half == 0 else slice(64, 128)
                    for n in range(4):
                        co = half * 2048 + n * 512
                        ps_x = pfc.tile([128, 512], F32, tag="fc", name="ps_x")
                        for k in range(4):
                            nc.tensor.matmul(
                                ps_x[:],
                                lhsT=embT[k][:, m * 128 : (m + 1) * 128],
                                rhs=Ww[k][:, co : co + 512],
                                start=(k == 0),
                                stop=(k == 3),
                            )
                        x_sb = st2.tile([128, 512], BF16, tag="xsb", bufs=4)
                        nc.vector.tensor_add(
                            x_sb[:], ps_x[:], enc_exp2[:, co : co + 512]
                        )
                        ns = slice(n * 512, (n + 1) * 512)
                        nc.scalar.dma_start(xpe[t0][xrow, ns], x_sb[0:64])
                        nc.scalar.dma_start(xpe[t1][xrow, ns], x_sb[64:128])

            def fc_piece(m, vc, fw_row):
                v0 = vc * 512
                hsL = st2.tile([128, 1024], BF16, tag="hsL", bufs=4)
                nc.gpsimd.dma_start(
                    hsL[:], hsT_d[m][:].rearrange("p k x -> p (k x)")
                )
                ps_f = pfc.tile([128, 512], F32, tag="fc", name="ps_f")
                for k in range(8):
                    nc.tensor.matmul(
                        ps_f[:],
                        lhsT=hsL[:, k * 128 : (k + 1) * 128],
                        rhs=fw_row[k][:],
                        start=(k == 0),
                        stop=(k == 7),
                    )
                o_sb = st2.tile([128, 512], F32, tag="fco", bufs=2)
                nc.vector.tensor_add(o_sb[:], ps_f[:], fcb_sb[:, v0 : v0 + 512])
                nc.scalar.dma_start(
                    logits[m * 128 : (m + 1) * 128, v0 : v0 + 512], o_sb[:]
                )

            # ---- LSTM with woven fillers
            h_sb = statep.tile([B, H], F32, tag="h")
            c_sb = statep.tile([B, H], F32, tag="c")
            nc.sync.dma_start(c_sb[:], c0[:])
            hT_prev = st2.tile([128, 512], BF16, tag="hT", bufs=2)
            for k in range(8):
                nc.scalar.dma_start(hT_prev[:, k * 64 : (k + 1) * 64], h0T[k])

            ident64 = constp.tile([64, 64], F32)
            from concourse.masks import make_identity

            make_identity(nc, ident64[:])

            xproj_m(0)
            xproj_m(1)
            xproj_m(2)
            next_fc_m = 0

            for t in range(T):
                xp = st2.tile([128, 2048], BF16, tag="xp", bufs=2)
                nc.sync.dma_start(xp[:], xpe[t][:])

                zt = []
                for n in range(4):
                    ps_zn = pz.tile([128, 512], F32, tag=f"z{n}", name=f"ps_zn{n}")
                    for k in range(8):
                        lhsT = hT_prev[:, k * 64 : (k + 1) * 64]
                        nc.tensor.matmul(
                            ps_zn[0:64, :],
                            lhsT=lhsT,
                            rhs=WhT[k][:, n * 512 : (n + 1) * 512],
                            start=(k == 0),
                            stop=(k == 7),
                            tile_position=(0, 0),
                            skip_group_check=True,
                        )
                        nc.tensor.matmul(
                            ps_zn[64:128, :],
                            lhsT=lhsT,
                            rhs=WhT[k][:, 2048 + n * 512 : 2048 + (n + 1) * 512],
                            start=(k == 0),
                            stop=(k == 7),
                            tile_position=(0, 64),
                            skip_group_check=True,
                        )
                    ns = slice(n * 512, (n + 1) * 512)
                    nc.vector.tensor_add(ps_zn[:], ps_zn[:], xp[:, ns])
                    nc.scalar.activation(ps_zn[0:64, :], ps_zn[0:64, :], Sig)
                    if n == 0:
                        go_sb = statep.tile([B, 2048], F32, tag="go", name="go_sb")
                    nc.scalar.activation(
                        go_sb[:, ns], ps_zn[64:128, :], Tanh if n < 2 else Sig
                    )
                    zt.append(ps_zn)

                # PE fillers while the c/h tail runs
                filled = False
                if t % 2 == 1:
                    mx = 3 + (t - 1) // 2
                    if mx <= 15:
                        xproj_m(mx)
                        filled = True
                if not filled and t >= 4 and next_fc_m <= min(15, (t - 2) // 2):
                    fc_piece(next_fc_m, 0, fwA)
                    next_fc_m += 1

                ps_tr = pfc.tile([128, 512], F32, tag="fc", name="ps_tr")
                for j in range(2):
                    js = slice(j * 512, (j + 1) * 512)
                    nc.vector.tensor_mul(c_sb[:, js], zt[2 + j][0:64, :], c_sb[:, js])
                    nc.vector.tensor_mul(go_sb[:, js], zt[j][0:64, :], go_sb[:, js])
                    nc.vector.tensor_add(c_sb[:, js], c_sb[:, js], go_sb[:, js])
                    nc.scalar.activation(go_sb[:, js], c_sb[:, js], Tanh)
                    nc.vector.tensor_mul(
                        h_sb[:, js],
                        go_sb[:, (2 + j) * 512 : (3 + j) * 512],
                        go_sb[:, js],
                    )
                    for k in range(4 * j, 4 * j + 4):
                        nc.tensor.transpose(
                            out=ps_tr[:, k * 64 : (k + 1) * 64],
                            in_=h_sb[:, k * 128 : (k + 1) * 128],
                            identity=ident64[:],
                        )
                hT_cur = st2.tile([128, 512], BF16, tag="hT", bufs=2)
                nc.vector.tensor_copy(hT_cur[:], ps_tr[:])
                m, hlf = t // 2, t % 2
                for k in range(8):
                    nc.sync.dma_start(
                        hsT_d[m][:, k, hlf * 64 : (hlf + 1) * 64],
                        hT_cur[:, k * 64 : (k + 1) * 64],
                    )
                hT_prev = hT_cur

            nc.sync.dma_start(hn[:], h_sb[:])
            nc.sync.dma_start(cn[:], c_sb[:])

            # ---- remaining FC: finish vc0, then chunks 1..7
            while next_fc_m <= 15:
                fc_piece(next_fc_m, 0, fwA)
                next_fc_m += 1
            for vc in range(1, 8):
                v0 = vc * 512
                fw = []
                for k in range(8):
                    fwk = st1.tile(
                        [128, 512], BF16, tag=f"fw{k}", name=f"fw{vc}_{k}", bufs=2
                    )
                    nc.gpsimd.dma_start(fwk[:], fcwT[k][:, v0 : v0 + 512])
                    fw.append(fwk)
                for m in range(16):
                    fc_piece(m, vc, fw)

    nc.compile()
    return nc


def get_nc() -> bass.Bass:
    global _nc_cache
    if _nc_cache is None:
        _nc_cache = build_nc()
    return _nc_cache


def _kt(a, k):
    """[K, N] -> [K/128, 128, N] bf16 contiguous."""
    K = a.shape[0]
    return np.ascontiguousarray(
        a.reshape(K // 128, 128, a.shape[1]).astype(BF16NP)
    )


def make_in_maps(inputs_np: dict) -> list:
    inp = {k: np.asarray(v) for k, v in inputs_np.items()}
    idx_tm = (
        np.ascontiguousarray(inp["inputs"].astype(np.int32).T)
        .reshape(16, 128, 1)
        .copy()
    )
    W_ih = np.asarray(inp["W_ih"], dtype=np.float32)
    bias = (
        np.asarray(inp["b_ih"], dtype=np.float32)
        + np.asarray(inp["b_hh"], dtype=np.float32)
    )
    enc_aug = np.zeros((EA, B), dtype=np.float32)
    enc_aug[:E] = np.asarray(inp["encoder_features"], dtype=np.float32).T
    enc_aug[E] = 1.0
    wenc_augT = np.zeros((EA, G4), dtype=np.float32)
    wenc_augT[:E] = W_ih[:, E:].T
    wenc_augT[E] = bias
    h0T = np.ascontiguousarray(np.asarray(inp["h0"], dtype=np.float32).T)
    common = dict(
        idx=idx_tm,
        emb=np.ascontiguousarray(inp["embed_table"], dtype=np.float32),
        c0=np.ascontiguousarray(inp["c0"], dtype=np.float32),
        h0T=_kt(h0T, 8),
        encT=_kt(enc_aug, 5),
        wihT=_kt(np.ascontiguousarray(W_ih[:, :E].T), 4),
        wencT=_kt(wenc_augT, 5),
        whhT=_kt(np.ascontiguousarray(np.asarray(inp["W_hh"], np.float32).T), 8),
    )
    fc_W = np.asarray(inp["fc_W"], dtype=np.float32)
    fc_b = np.asarray(inp["fc_b"], dtype=np.float32)
    in_maps = []
    for i in range(NCORES):
        fcw_pad = np.zeros((H, VLP), dtype=np.float32)
        fcw_pad[:, :VL] = fc_W[i * VL : (i + 1) * VL].T
        fcb_pad = np.zeros((1, VLP), dtype=np.float32)
        fcb_pad[0, :VL] = fc_b[i * VL : (i + 1) * VL]
        fcb_rep = np.broadcast_to(fcb_pad, (128, VLP)).astype(BF16NP)
        in_maps.append(
            dict(common, fcwT=_kt(fcw_pad, 8), fcb=np.ascontiguousarray(fcb_rep))
        )
    return in_maps


def assemble(results: list):
    logits_tm = np.concatenate(
        [results[i]["logits"][:, :VL] for i in range(NCORES)], axis=1
    )  # [BT(t-major), V]
    logits = np.ascontiguousarray(
        logits_tm.reshape(T, B, V).transpose(1, 0, 2)
    )  # [B, T, V]
    hn = results[0]["hn"]
    cn = results[0]["cn"]
    return logits, hn, cn


def run(inputs_np: dict, trace: bool = False):
    nc = get_nc()
    in_maps = make_in_maps(inputs_np)
    res = run_bass_kernel_spmd(
        nc, in_maps, core_ids=list(range(NCORES)), trace=trace
    )
    return assemble(res.results), res


def kernel(**inputs) -> tuple:
    (logits, hn, cn), _ = run(inputs, trace=False)
    return logits, hn, cn


# revision 26
# speedup vs baseline: 1.1186x; 1.0893x over previous
"""Trainium2 Bass kernel for DecoderRNN (embed -> LSTM -> vocab FC).

Strategy (8 NeuronCores, SPMD):
  - Embedding gather, x_proj precompute and the LSTM recurrence are
    replicated on every core (per-step cross-core collectives are
    latency-bound and slower than replicating ~7us/step of matmul).
  - The dominant FC (hs @ fc_W.T, 134 of 160 GFLOP) is sharded along the
    vocab dim: each core gets 4000 rows of fc_W (zero-padded to 4096) and
    produces logits[:, shard]. Host concatenates.
  - All weight matrices are pre-transposed/cast to bf16 on the host into
    the [K-tile, 128, N] layouts the PE consumes; only the data-dependent
    embedding gather is transposed on device (via bf16 DMA-transpose).
  - Token order is t-major everywhere (bt = t*64 + b).
  - The recurrence matmul is column-tiled: batch (M=64) fills PE columns
    0:63 for gate cols [i|f] and columns 64:127 for [g|o] concurrently,
    one single-bank PSUM tile per 512-slice; gates are slice-pipelined
    under the matmul stream (ACT moves the g|o half PSUM->SBUF for free).
  - x_proj m-tiles and FC pieces are woven between LSTM steps to keep the
    PE busy through the recurrence's gate latency.
  - b_ih+b_hh folded into the encoder projection via K-augmentation.
"""

import os
import sys

import numpy as np

for _p in ("/opt/trn_rl_repo", "/root/.axon_site/_ro/trn_rl_repo"):
    if os.path.isdir(_p) and _p not in sys.path:
        sys.path.append(_p)

import ml_dtypes
import concourse.bass as bass
import concourse.mybir as mybir
from concourse import bacc
import concourse.tile as tile
from concourse.bass_utils import run_bass_kernel_spmd

F32 = mybir.dt.float32
BF16 = mybir.dt.bfloat16
I32 = mybir.dt.int32
BF16NP = ml_dtypes.bfloat16

B, T = 64, 32
E, H, V = 512, 1024, 32000
G4 = 4 * H          # 4096
BT = B * T          # 2048
EA = E + 128        # augmented enc K dim (ones col + zero pad)
NCORES = 8
VL = V // NCORES    # 4000 real vocab rows per core
VLP = 4096          # padded vocab rows per core

Sig = mybir.ActivationFunctionType.Sigmoid
Tanh = mybir.ActivationFunctionType.Tanh

_nc_cache = None


def build_nc() -> bass.Bass:
    nc = bacc.Bacc()

    idx = nc.declare_dram_parameter("idx", [16, 128, 1], I32, isOutput=False)
    emb = nc.declare_dram_parameter("emb", [V, E], F32, isOutput=False)
    c0 = nc.declare_dram_parameter("c0", [B, H], F32, isOutput=False)
    h0T = nc.declare_dram_parameter("h0T", [8, 128, B], BF16, isOutput=False)
    encT = nc.declare_dram_parameter("encT", [5, 128, B], BF16, isOutput=False)
    wihT = nc.declare_dram_parameter("wihT", [4, 128, G4], BF16, isOutput=False)
    wencT = nc.declare_dram_parameter("wencT", [5, 128, G4], BF16, isOutput=False)
    whhT = nc.declare_dram_parameter("whhT", [8, 128, G4], BF16, isOutput=False)
    fcwT = nc.declare_dram_parameter("fcwT", [8, 128, VLP], BF16, isOutput=False)
    fcb = nc.declare_dram_parameter("fcb", [128, VLP], BF16, isOutput=False)

    logits = nc.declare_dram_parameter("logits", [BT, VLP], F32, isOutput=True)
    hn = nc.declare_dram_parameter("hn", [B, H], F32, isOutput=True)
    cn = nc.declare_dram_parameter("cn", [B, H], F32, isOutput=True)

    with tile.TileContext(nc) as tc:
        with (
            tc.tile_pool(name="dram", bufs=1, space="DRAM") as dr,
            tc.tile_pool(name="constp", bufs=1) as constp,
            tc.tile_pool(name="wres", bufs=1) as wres,
            tc.tile_pool(name="st1", bufs=1) as st1,
            tc.tile_pool(name="st2", bufs=1) as st2,
            tc.tile_pool(name="statep", bufs=1) as statep,
            tc.tile_pool(name="pz", bufs=1, space="PSUM") as pz,
            tc.tile_pool(name="pfc", bufs=4, space="PSUM") as pfc,
        ):
            # ---- DRAM scratch
            emb_bf = [
                dr.tile([128, E], BF16, name=f"emb_bf{m}") for m in range(16)
            ]
            xpe = [dr.tile([128, 2048], BF16, name=f"xpe{t}") for t in range(T)]
            hsT_d = [
                dr.tile([128, 8, 128], BF16, name=f"hsT_d{m}") for m in range(16)
            ]

            # ---- embedding gather (t-major), cast bf16, transpose
            for m in range(16):
                idx_sb = st2.tile([128, 1], I32, tag="idx", bufs=2)
                nc.sync.dma_start(idx_sb[:], idx[m])
                g_f = st2.tile([128, E], F32, tag="gf", bufs=1)
                nc.gpsimd.indirect_dma_start(
                    out=g_f[:],
                    out_offset=None,
                    in_=emb[:],
                    in_offset=bass.IndirectOffsetOnAxis(ap=idx_sb[:, :1], axis=0),
                )
                g_b = st2.tile([128, E], BF16, tag="gb", bufs=2)
                nc.vector.tensor_copy(g_b[:], g_f[:])
                nc.sync.dma_start(emb_bf[m][:], g_b[:])
            embT = []
            for k in range(4):
                tT = st1.tile([128, BT], BF16, tag=f"embT{k}", name=f"embT{k}")
                embT.append(tT)
            for m in range(16):
                for k in range(4):
                    nc.sync.dma_start_transpose(
                        embT[k][:, m * 128 : (m + 1) * 128],
                        emb_bf[m][:, k * 128 : (k + 1) * 128],
                    )

            # ---- enc_proj (+bias via augmentation) -> enc_exp2 bf16
            encT_sb = []
            for k in range(5):
                tT = st2.tile([128, B], BF16, tag=f"encT{k}", name=f"encT{k}")
                nc.scalar.dma_start(tT[:], encT[k])
                encT_sb.append(tT)
            enc_exp2 = wres.tile([128, G4], BF16)
            for half in range(2):
                hoff = half * 2048
                ps_es = [
                    pfc.tile([64, 512], F32, tag="fc", name=f"ps_e{n}")
                    for n in range(4)
                ]
                for k in range(5):
                    weT = st2.tile([128, 2048], BF16, tag="WeT", bufs=2)
                    nc.scalar.dma_start(weT[:], wencT[k][:, hoff : hoff + 2048])
                    for n in range(4):
                        nc.tensor.matmul(
                            ps_es[n][:],
                            lhsT=encT_sb[k][:],
                            rhs=weT[:, n * 512 : (n + 1) * 512],
                            start=(k == 0),
                            stop=(k == 4),
                        )
                for n in range(4):
                    nc.vector.tensor_copy(
                        enc_exp2[0:64, hoff + n * 512 : hoff + (n + 1) * 512],
                        ps_es[n][:],
                    )
            nc.sync.dma_start(enc_exp2[64:128], enc_exp2[0:64])

            # ---- resident weights, spread across DMA engines
            engs = [nc.sync, nc.scalar, nc.gpsimd]
            Ww = []
            for k in range(4):
                tT = st1.tile([128, G4], BF16, tag=f"Ww{k}", name=f"Ww{k}")
                engs[k % 2].dma_start(tT[:], wihT[k])
                Ww.append(tT)
            WhT = []
            for k in range(8):
                tT = wres.tile([128, G4], BF16, tag=f"WhT{k}", name=f"WhT{k}")
                engs[k % 3].dma_start(tT[:], whhT[k])
                WhT.append(tT)
            fwA = []
            for k in range(8):
                fwk = st1.tile([128, 512], BF16, tag=f"fwA{k}", name=f"fwA{k}")
                nc.gpsimd.dma_start(fwk[:], fcwT[k][:, 0:512])
                fwA.append(fwk)
            fcb_sb = constp.tile([128, VLP], BF16)
            nc.scalar.dma_start(fcb_sb[:], fcb[:])

            # ---- program pieces
            # gate-column pairing: psum slice s holds top=T(s), bottom=BOT(s)
            # s: 0=(i0,g0) 1=(f0,o0) 2=(i1,g1) 3=(f1,o1) -> c-half j chains
            # start as soon as slices 2j, 2j+1 are done
            TOPC = [0, 1024, 512, 1536]           # i0 f0 i1 f1
            BOTC = [2048, 3072, 2560, 3584]       # g0 o0 g1 o1

            def xproj_m(m):
                """x_proj for tokens t0=2m, t1=2m+1 -> xpe[t0], xpe[t1]."""
                t0, t1 = 2 * m, 2 * m + 1
                for s in range(4):
                    for top in (True, False):
                        co = TOPC[s] if top else BOTC[s]
                        xrow = slice(0, 64) if top else slice(64, 128)
                        ps_x = pfc.tile([128, 512], F32, tag="fc", name="ps_x")
                        for k in range(4):
                            nc.tensor.matmul(
                                ps_x[:],
                                lhsT=embT[k][:, m * 128 : (m + 1) * 128],
                                rhs=Ww[k][:, co : co + 512],
                                start=(k == 0),
                                stop=(k == 3),
                            )
                        x_sb = st2.tile([128, 512], BF16, tag="xsb", bufs=4)
                        nc.vector.tensor_add(
                            x_sb[:], ps_x[:], enc_exp2[:, co : co + 512]
                        )
                        ns = slice(s * 512, (s + 1) * 512)
                        nc.scalar.dma_start(xpe[t0][xrow, ns], x_sb[0:64])
                        nc.scalar.dma_start(xpe[t1][xrow, ns], x_sb[64:128])

            def fc_piece(m, vc, fw_row):
                v0 = vc * 512
                hsL = st2.tile([128, 1024], BF16, tag="hsL", bufs=4)
                nc.gpsimd.dma_start(
                    hsL[:], hsT_d[m][:].rearrange("p k x -> p (k x)")
                )
                ps_f = pfc.tile([128, 512], F32, tag="fc", name="ps_f")
                for k in range(8):
                    nc.tensor.matmul(
                        ps_f[:],
                        lhsT=hsL[:, k * 128 : (k + 1) * 128],
                        rhs=fw_row[k][:],
                        start=(k == 0),
                        stop=(k == 7),
                    )
                o_sb = st2.tile([128, 512], F32, tag="fco", bufs=2)
                nc.vector.tensor_add(o_sb[:], ps_f[:], fcb_sb[:, v0 : v0 + 512])
                nc.scalar.dma_start(
                    logits[m * 128 : (m + 1) * 128, v0 : v0 + 512], o_sb[:]
                )

            # ---- LSTM with woven fillers
            h_sb = statep.tile([B, H], F32, tag="h")
            c_sb = statep.tile([B, H], F32, tag="c")
            nc.sync.dma_start(c_sb[:], c0[:])
            hTa_prev = st2.tile([128, 256], BF16, tag="hTa", bufs=2, name="hTa0")
            hTb_prev = st2.tile([128, 256], BF16, tag="hTb", bufs=2, name="hTb0")
            for k in range(8):
                dst = hTa_prev if k < 4 else hTb_prev
                nc.scalar.dma_start(dst[:, (k % 4) * 64 : (k % 4 + 1) * 64], h0T[k])

            ident64 = constp.tile([64, 64], F32)
            from concourse.masks import make_identity

            make_identity(nc, ident64[:])

            xproj_m(0)
            xproj_m(1)
            xproj_m(2)
            next_fc_m = 0

            for t in range(T):
                xp = st2.tile([128, 2048], BF16, tag="xp", bufs=2)
                nc.sync.dma_start(xp[:], xpe[t][:])

                zt = [
                    pz.tile([128, 512], F32, tag=f"z{n}", name=f"ps_zn{n}")
                    for n in range(4)
                ]
                for kphase in range(2):
                    for s in range(4):
                        ps_zn = zt[s]
                        hT_half = hTa_prev if kphase == 0 else hTb_prev
                        for k in range(4 * kphase, 4 * kphase + 4):
                            lhsT = hT_half[:, (k % 4) * 64 : (k % 4 + 1) * 64]
                            nc.tensor.matmul(
                                ps_zn[0:64, :],
                                lhsT=lhsT,
                                rhs=WhT[k][:, TOPC[s] : TOPC[s] + 512],
                                start=(k == 0),
                                stop=(k == 7),
                                tile_position=(0, 0),
                                skip_group_check=True,
                            )
                            nc.tensor.matmul(
                                ps_zn[64:128, :],
                                lhsT=lhsT,
                                rhs=WhT[k][:, BOTC[s] : BOTC[s] + 512],
                                start=(k == 0),
                                stop=(k == 7),
                                tile_position=(0, 64),
                                skip_group_check=True,
                            )
                        if kphase == 1:
                            ns = slice(s * 512, (s + 1) * 512)
                            nc.vector.tensor_add(ps_zn[:], ps_zn[:], xp[:, ns])
                            nc.scalar.activation(ps_zn[0:64, :], ps_zn[0:64, :], Sig)
                            if s == 0:
                                go_sb = statep.tile(
                                    [B, 2048], F32, tag="go", name="go_sb"
                                )
                            cb = (s % 2) * 2 + s // 2  # go cols [g0,g1,o0,o1]
                            nc.scalar.activation(
                                go_sb[:, cb * 512 : (cb + 1) * 512],
                                ps_zn[64:128, :],
                                Tanh if s % 2 == 0 else Sig,
                            )

                # PE fillers while the c/h tail runs
                filled = False
                if t % 2 == 1:
                    mx = 3 + (t - 1) // 2
                    if mx <= 15:
                        xproj_m(mx)
                        filled = True
                if not filled and t >= 4 and next_fc_m <= min(15, (t - 2) // 2):
                    fc_piece(next_fc_m, 0, fwA)
                    next_fc_m += 1

                ps_tr = pfc.tile([128, 512], F32, tag="fc", name="ps_tr")
                hTa_cur = st2.tile([128, 256], BF16, tag="hTa", bufs=2, name="hTa")
                hTb_cur = st2.tile([128, 256], BF16, tag="hTb", bufs=2, name="hTb")
                m, hlf = t // 2, t % 2
                for j in range(2):
                    js = slice(j * 512, (j + 1) * 512)
                    # go_sb cols: [g0, g1, o0, o1]; g_j at 512j, o_j at 512(2+j)
                    nc.vector.tensor_mul(
                        c_sb[:, js], zt[2 * j + 1][0:64, :], c_sb[:, js]
                    )
                    nc.vector.tensor_mul(go_sb[:, js], zt[2 * j][0:64, :], go_sb[:, js])
                    nc.vector.tensor_add(c_sb[:, js], c_sb[:, js], go_sb[:, js])
                    nc.scalar.activation(go_sb[:, js], c_sb[:, js], Tanh)
                    nc.vector.tensor_mul(
                        h_sb[:, js],
                        go_sb[:, (2 + j) * 512 : (3 + j) * 512],
                        go_sb[:, js],
                    )
                    for k in range(4 * j, 4 * j + 4):
                        nc.tensor.transpose(
                            out=ps_tr[:, k * 64 : (k + 1) * 64],
                            in_=h_sb[:, k * 128 : (k + 1) * 128],
                            identity=ident64[:],
                        )
                    hT_half = hTa_cur if j == 0 else hTb_cur
                    nc.vector.tensor_copy(
                        hT_half[:], ps_tr[:, j * 256 : (j + 1) * 256]
                    )
                    for k in range(4 * j, 4 * j + 4):
                        nc.sync.dma_start(
                            hsT_d[m][:, k, hlf * 64 : (hlf + 1) * 64],
                            hT_half[:, (k % 4) * 64 : (k % 4 + 1) * 64],
                        )
                hTa_prev, hTb_prev = hTa_cur, hTb_cur

            nc.sync.dma_start(hn[:], h_sb[:])
            nc.sync.dma_start(cn[:], c_sb[:])

            # ---- remaining FC: finish vc0, then chunks 1..7
            while next_fc_m <= 15:
                fc_piece(next_fc_m, 0, fwA)
                next_fc_m += 1
            for vc in range(1, 8):
                v0 = vc * 512
                fw = []
                for k in range(8):
                    fwk = st1.tile(
                        [128, 512], BF16, tag=f"fw{k}", name=f"fw{vc}_{k}", bufs=2
                    )
                    nc.gpsimd.dma_start(fwk[:], fcwT[k][:, v0 : v0 + 512])
                    fw.append(fwk)
                for m in range(16):
                    fc_piece(m, vc, fw)

    nc.compile()
    return nc


def get_nc() -> bass.Bass:
    global _nc_cache
    if _nc_cache is None:
        _nc_cache = build_nc()
    return _nc_cache


def _kt(a, k):
    """[K, N] -> [K/128, 128, N] bf16 contiguous."""
    K = a.shape[0]
    return np.ascontiguousarray(
        a.reshape(K // 128, 128, a.shape[1]).astype(BF16NP)
    )


def make_in_maps(inputs_np: dict) -> list:
    inp = {k: np.asarray(v) for k, v in inputs_np.items()}
    idx_tm = (
        np.ascontiguousarray(inp["inputs"].astype(np.int32).T)
        .reshape(16, 128, 1)
        .copy()
    )
    W_ih = np.asarray(inp["W_ih"], dtype=np.float32)
    bias = (
        np.asarray(inp["b_ih"], dtype=np.float32)
        + np.asarray(inp["b_hh"], dtype=np.float32)
    )
    enc_aug = np.zeros((EA, B), dtype=np.float32)
    enc_aug[:E] = np.asarray(inp["encoder_features"], dtype=np.float32).T
    enc_aug[E] = 1.0
    wenc_augT = np.zeros((EA, G4), dtype=np.float32)
    wenc_augT[:E] = W_ih[:, E:].T
    wenc_augT[E] = bias
    h0T = np.ascontiguousarray(np.asarray(inp["h0"], dtype=np.float32).T)
    common = dict(
        idx=idx_tm,
        emb=np.ascontiguousarray(inp["embed_table"], dtype=np.float32),
        c0=np.ascontiguousarray(inp["c0"], dtype=np.float32),
        h0T=_kt(h0T, 8),
        encT=_kt(enc_aug, 5),
        wihT=_kt(np.ascontiguousarray(W_ih[:, :E].T), 4),
        wencT=_kt(wenc_augT, 5),
        whhT=_kt(np.ascontiguousarray(np.asarray(inp["W_hh"], np.float32).T), 8),
    )
    fc_W = np.asarray(inp["fc_W"], dtype=np.float32)
    fc_b = np.asarray(inp["fc_b"], dtype=np.float32)
    in_maps = []
    for i in range(NCORES):
        fcw_pad = np.zeros((H, VLP), dtype=np.float32)
        fcw_pad[:, :VL] = fc_W[i * VL : (i + 1) * VL].T
        fcb_pad = np.zeros((1, VLP), dtype=np.float32)
        fcb_pad[0, :VL] = fc_b[i * VL : (i + 1) * VL]
        fcb_rep = np.broadcast_to(fcb_pad, (128, VLP)).astype(BF16NP)
        in_maps.append(
            dict(common, fcwT=_kt(fcw_pad, 8), fcb=np.ascontiguousarray(fcb_rep))
        )
    return in_maps


def assemble(results: list):
    logits_tm = np.concatenate(
        [results[i]["logits"][:, :VL] for i in range(NCORES)], axis=1
    )  # [BT(t-major), V]
    logits = np.ascontiguousarray(
        logits_tm.reshape(T, B, V).transpose(1, 0, 2)
    )  # [B, T, V]
    hn = results[0]["hn"]
    cn = results[0]["cn"]
    return logits, hn, cn


def run(inputs_np: dict, trace: bool = False):
    nc = get_nc()
    in_maps = make_in_maps(inputs_np)
    res = run_bass_kernel_spmd(
        nc, in_maps, core_ids=list(range(NCORES)), trace=trace
    )
    return assemble(res.results), res


def kernel(**inputs) -> tuple:
    (logits, hn, cn), _ = run(inputs, trace=False)
    return logits, hn, cn
